# revision 3
# baseline (speedup 1.0000x reference)
"""Trainium2 Bass kernel for masked-pool + per-sample expert matmul (moe_routing).

Computation (reference):
    attended[b,c] = mean_hw(mask[b,hw] * features[b,c,hw])        # [B,C]
    preds[b,a]    = sum_c attended[b,c] * weight[inst[b],c,a] + bias[inst[b],a]

Sharding: expert-parallel with host-side routing. The 32 experts are packed
into 8 bins of 4 (balanced by sample count); each core gets the features of
the samples routed to its 4 experts (padded to S rows), its 4 experts'
weights, and an indicator matrix ind[slot, row] = 1/196 marking which rows
belong to which expert slot. On device, each slot's matmul uses the
indicator-masked attended matrix as the stationary operand, accumulating all
4 slots into one PSUM tile, so each core reads only its own 4 experts'
weights (16.4MB) + its own samples' features (~16MB) -- near the HBM
roofline for this memory-bound problem.
"""

import numpy as np

import concourse.bacc as bacc
import concourse.tile as tile
from concourse import mybir
from concourse.bass_utils import run_bass_kernel_spmd

B, C, H, W = 256, 512, 14, 14
HWD = H * W  # 196
N_EXP, N_ANS = 32, 2000
N_CORES = 8
E = N_EXP // N_CORES  # expert slots per core = 4
S_DEFAULT = 40        # padded samples per core (max balanced bin load + margin)
J = C // 128          # c-chunks = 4
N_TILE = 512
NT = (N_ANS + N_TILE - 1) // N_TILE  # 4 (2000 = 3*512 + 464)

_compiled = {}  # S -> (nc, names)


def _build(S):
    fp32 = mybir.dt.float32
    nc = bacc.Bacc("TRN2", target_bir_lowering=False, debug=False,
                   num_devices=N_CORES)
    feat = nc.dram_tensor("feat", [S, C, HWD], fp32, kind="ExternalInput")
    maskv = nc.dram_tensor("maskv", [1, S, HWD], fp32, kind="ExternalInput")
    wt = nc.dram_tensor("wt", [E, C, N_ANS], fp32, kind="ExternalInput")
    bsg = nc.dram_tensor("bsg", [S, N_ANS], fp32, kind="ExternalInput")
    ind = nc.dram_tensor("ind", [1, E, S], fp32, kind="ExternalInput")
    out = nc.dram_tensor("out", [S, N_ANS], fp32, kind="ExternalOutput")

    with tile.TileContext(nc) as tc:
        with (
            tc.tile_pool(name="persist", bufs=1) as persist,
            tc.tile_pool(name="fpool", bufs=4) as fpool,
            tc.tile_pool(name="ppool", bufs=4) as ppool,
            tc.tile_pool(name="spool", bufs=2) as spool,
            tc.tile_pool(name="wpool", bufs=8) as wpool,
            tc.tile_pool(name="psum", bufs=2, space="PSUM") as psum_pool,
        ):
            # broadcast mask + indicator rows across all 128 partitions
            mb = persist.tile([128, S, HWD], fp32)
            nc.sync.dma_start(mb[:], maskv.ap().to_broadcast((128, S, HWD)))
            indb = persist.tile([128, E, S], fp32)
            nc.sync.dma_start(indb[:], ind.ap().to_broadcast((128, E, S)))

            attT = persist.tile([128, J, S], fp32)   # attended^T (unscaled)
            # phase 1: attT[c,j,i] = sum_hw feat[i, j*128+c, hw] * mask[i,hw]
            for i in range(S):
                ft = fpool.tile([128, J, HWD], fp32, tag="ft")
                nc.gpsimd.dma_start(
                    ft[:], feat.ap()[i].rearrange("(j p) h -> p j h", p=128))
                for j in range(J):
                    pr = ppool.tile([128, HWD], fp32, tag="pr")
                    nc.vector.tensor_mul(pr[:], ft[:, j, :], mb[:, i, :])
                    sc = spool.tile([128, HWD], fp32, tag="sc")
                    nc.scalar.activation(
                        sc[:], pr[:], mybir.ActivationFunctionType.Copy,
                        accum_out=attT[:, j, i:i + 1])

            # indicator mask (also folds in the 1/196 mean scaling)
            matt = persist.tile([128, E, J, S], fp32)
            for g in range(E):
                for j in range(J):
                    nc.vector.tensor_mul(
                        matt[:, g, j, :], attT[:, j, :], indb[:, g, :])

            bias_sb = persist.tile([S, N_ANS], fp32)
            nc.sync.dma_start(bias_sb[:], bsg.ap())
            out_sb = persist.tile([S, N_ANS], fp32)

            # phase 2: out[i,a] = sum_g sum_c matt[c,g,i] * wt[g,c,a] + bias
            for nt in range(NT):
                n0 = nt * N_TILE
                n1 = min(N_ANS, n0 + N_TILE)
                ps = psum_pool.tile([S, N_TILE], fp32, tag="ps")
                k = 0
                for g in range(E):
                    for j in range(J):
                        wtile = wpool.tile([128, N_TILE], fp32, tag="wt")
                        nc.sync.dma_start(
                            wtile[:, :n1 - n0],
                            wt.ap()[g, j * 128:(j + 1) * 128, n0:n1])
                        nc.tensor.matmul(
                            ps[:, :n1 - n0], matt[:, g, j, :],
                            wtile[:, :n1 - n0],
                            start=(k == 0), stop=(k == E * J - 1))
                        k += 1
                nc.vector.tensor_add(
                    out_sb[:, n0:n1], ps[:, :n1 - n0], bias_sb[:, n0:n1])
            nc.sync.dma_start(out.ap(), out_sb[:])
    nc.compile()
    return nc


def _get_compiled(S):
    if S not in _compiled:
        _compiled[S] = _build(S)
    return _compiled[S]


def _route(instance, S):
    """Pack 32 experts into 8 bins of 4, balanced by sample count.

    Returns (bins, sample_lists): bins[c] = 4 expert ids, sample_lists[c] =
    sample indices routed to core c (grouped by expert slot order).
    """
    cnt = np.bincount(instance, minlength=N_EXP)
    order = np.argsort(-cnt, kind="stable")
    bins = [[] for _ in range(N_CORES)]
    loads = [0] * N_CORES
    for e in order:
        cands = [b for b in range(N_CORES) if len(bins[b]) < E]
        b = min(cands, key=lambda x: loads[x])
        bins[b].append(int(e))
        loads[b] += int(cnt[e])
    sample_lists = []
    for c in range(N_CORES):
        samp = np.concatenate(
            [np.where(instance == e)[0] for e in bins[c]])
        sample_lists.append(samp)
    return bins, sample_lists, max(loads)


def kernel(mask, features, weight, bias, instance):
    mask = np.ascontiguousarray(np.asarray(mask, dtype=np.float32))
    features = np.ascontiguousarray(np.asarray(features, dtype=np.float32))
    weight = np.ascontiguousarray(np.asarray(weight, dtype=np.float32))
    bias = np.ascontiguousarray(np.asarray(bias, dtype=np.float32))
    inst = np.asarray(instance).astype(np.int64)

    b = features.shape[0]
    assert features.shape == (B, C, H, W) and b == B

    S = S_DEFAULT
    bins, sample_lists, max_load = _route(inst, S)
    while max_load > S:  # fallback for pathological routing distributions
        S = ((max_load + 7) // 8) * 8
        bins, sample_lists, max_load = _route(inst, S)

    nc = _get_compiled(S)

    feat_flat = features.reshape(B, C, HWD)
    mask_flat = mask.reshape(B, HWD)
    in_maps = []
    for c in range(N_CORES):
        samp = sample_lists[c]
        n_c = len(samp)
        pad_to = S - n_c
        if n_c > 0:
            padded = np.concatenate([samp, np.full(pad_to, samp[0])])
        else:
            padded = np.zeros(S, dtype=np.int64)
        ind_c = np.zeros((1, E, S), dtype=np.float32)
        slot_of = {e: g for g, e in enumerate(bins[c])}
        for k in range(n_c):
            ind_c[0, slot_of[int(inst[samp[k]])], k] = 1.0 / HWD
        in_maps.append({
            "feat": np.ascontiguousarray(feat_flat[padded]),
            "maskv": np.ascontiguousarray(mask_flat[padded])[None],
            "wt": np.ascontiguousarray(weight[bins[c]]),
            "bsg": np.ascontiguousarray(bias[inst[padded]]),
            "ind": ind_c,
        })

    res = run_bass_kernel_spmd(nc, in_maps, list(range(N_CORES)))

    preds = np.empty((B, N_ANS), dtype=np.float32)
    for c in range(N_CORES):
        samp = sample_lists[c]
        preds[samp] = res.results[c]["out"][:len(samp)]
    return preds
